# revision 10
# baseline (speedup 1.0000x reference)
"""Causal self-attention kernel for Trainium2, 8-way sharded.

Problem: B=2, T=2048, C=1024, NH=16, hd=64. fp32 in/out.

Sharding: core = (batch b, head-group g of 4 heads). Each core computes its
4 heads' attention for its batch plus the partial output projection
y_local @ Wo[g*256:(g+1)*256, :]; the host sums the 4 partials per batch
(biases bv/bo are folded in exactly via a host-side correction row).

v2 schedule (vs baseline):
  - All matmul tensors bf16 (x, weights, q/k, P, V, y): halves DMA both
    ways, enables FWL weight loads. Scores/projections accumulate fp32 in
    PSUM so only operand quantization is lost (~0.5%, budget is 2e-2).
  - x is DMAed T-slice-major (slice = 512 tq x all 8 C-chunks = 0.5MB)
    so projections stream as slices land; PE is busy from ~4us and HAM
    stays warm instead of idling 24us for the full-x DMA.
  - Window-major over head-pairs: for each tq window w, both pairs' score/
    exp/PV run back-to-back, so the ScalarE exp stream never starves
    (baseline had a 42us ScalarE hole between pairs).
  - Per-window normalization: denominators for the 4 (pair,head) lanes of
    window w are batched into one [4,512] DVE reciprocal; the output
    projection for window w then runs as PE filler inside window w+1's
    (ACT-bound) attention instead of a 64us serialized tail.
  - Output DMAed as bf16 partials; host sums in float64.
"""
import contextlib

import ml_dtypes
import numpy as np

import concourse.bass as bass
import concourse.tile as tile
from concourse import bacc, mybir
from concourse import bass_utils

bass_utils.upload_artifacts = lambda tmpdir: "local://skipped"

B, T, C = 2, 2048, 1024
NH, HD = 16, 64
NHL = 4            # heads per core
CLOC = NHL * HD    # 256 local channels
NCH = C // 128     # 8 contraction chunks
TQW = 512          # tq window / T-slice width
NW = T // TQW      # 4 windows
NTT = T // 128     # 16 t-tiles / tk-chunks
VSTR = HD + 8      # 72: v cols per head + 8 ones cols (denom at row 64)
LAG = 2            # PV trails S^T by this many chunk-groups
F32R = mybir.dt.float32r
F32 = mybir.dt.float32
BF16 = mybir.dt.bfloat16

_cache = {}


def _build():
    nc = bacc.Bacc("TRN2", target_bir_lowering=False, debug=False, num_devices=8)

    # x slice-major: slice s holds chunks c at offset (s*NCH+c)*TQW
    xt_ap = nc.dram_tensor("xt", [128, NW * NCH * TQW], BF16, kind="ExternalInput").ap()
    wq_ap = nc.dram_tensor("wq", [128, 2 * NCH * 128], BF16, kind="ExternalInput").ap()
    wk_ap = nc.dram_tensor("wk", [128, 2 * NCH * 128], BF16, kind="ExternalInput").ap()
    wv_ap = nc.dram_tensor("wv", [128, NCH * CLOC], BF16, kind="ExternalInput").ap()
    wo_ap = nc.dram_tensor("wo", [128, 2 * C], BF16, kind="ExternalInput").ap()
    bq_ap = nc.dram_tensor("bq", [2, 128, 1], F32, kind="ExternalInput").ap()
    bk_ap = nc.dram_tensor("bk", [2, 128, 1], F32, kind="ExternalInput").ap()
    ones_ap = nc.dram_tensor("ones", [128, NTT, NHL, 8], BF16, kind="ExternalInput").ap()
    selw_ap = nc.dram_tensor("selw", [128, 2 * 128], F32, kind="ExternalInput").ap()
    tri_ap = nc.dram_tensor("tri", [128, 128], BF16, kind="ExternalInput").ap()
    out_ap = nc.dram_tensor("out", [T, C], BF16, kind="ExternalOutput").ap()

    with tile.TileContext(nc) as tc, contextlib.ExitStack() as ctx:
        sb = ctx.enter_context(tc.tile_pool(name="sb", bufs=1))
        ost_pool = ctx.enter_context(tc.tile_pool(name="ost", bufs=3))
        pt_pool = ctx.enter_context(tc.tile_pool(name="ptp", bufs=10))
        ps = ctx.enter_context(tc.tile_pool(name="ps", bufs=1, space="PSUM"))

        # ---- persistent SBUF tensors ----
        wqs = sb.tile([128, 2 * NCH * 128], BF16, tag="wqs")
        wks = sb.tile([128, 2 * NCH * 128], BF16, tag="wks")
        wvs = sb.tile([128, NCH * CLOC], BF16, tag="wvs")
        wos = sb.tile([128, 2 * C], BF16, tag="wos")
        xts = sb.tile([128, NW * NCH * TQW], BF16, tag="xts")
        qts = [sb.tile([128, T], BF16, tag=f"qt{p}", name=f"qt{p}") for p in range(2)]
        kts = [sb.tile([128, T], BF16, tag=f"kt{p}", name=f"kt{p}") for p in range(2)]
        vna = sb.tile([128, NTT * NHL * VSTR], BF16, tag="vna")
        yts = [sb.tile([128, T], BF16, tag=f"yt{p}", name=f"yt{p}") for p in range(2)]
        bqs = [sb.tile([128, 1], F32, tag=f"bq{p}", name=f"bqs{p}") for p in range(2)]
        bks = [sb.tile([128, 1], F32, tag=f"bk{p}", name=f"bks{p}") for p in range(2)]
        selw = sb.tile([128, 2 * 128], F32, tag="selw")
        dsb = sb.tile([128, TQW], F32, tag="dsb")
        nc.vector.memset(dsb[:], 1.0)
        tri = sb.tile([128, 128], BF16, tag="tri")
        rcoll = sb.tile([128, TQW], F32, tag="rcoll")

        # ---- input DMAs ----
        # scalar queue: tiny constants + w weights (all land < ~5us, before
        # the exp stream owns ScalarE). sync: x slices 0,1. gpsimd: wv/ones
        # then x slices 2,3 and wo.
        for p in range(2):
            nc.scalar.dma_start(bqs[p][:], bq_ap[p])
            nc.scalar.dma_start(bks[p][:], bk_ap[p])
        HW = NCH * 128  # 1024 cols = one pair's weight block
        nc.scalar.dma_start(wqs[:, 0:HW], wq_ap[:, 0:HW])
        nc.scalar.dma_start(wks[:, 0:HW], wk_ap[:, 0:HW])
        nc.scalar.dma_start(wqs[:, HW:2 * HW], wq_ap[:, HW:2 * HW])
        nc.scalar.dma_start(wks[:, HW:2 * HW], wk_ap[:, HW:2 * HW])
        nc.scalar.dma_start(selw[:], selw_ap[:])
        nc.scalar.dma_start(tri[:], tri_ap[:])
        nc.sync.dma_start(xts[:, 0:NCH * TQW], xt_ap[:, 0:NCH * TQW])
        nc.sync.dma_start(xts[:, NCH * TQW:2 * NCH * TQW],
                          xt_ap[:, NCH * TQW:2 * NCH * TQW])
        nc.sync.dma_start(xts[:, 2 * NCH * TQW:3 * NCH * TQW],
                          xt_ap[:, 2 * NCH * TQW:3 * NCH * TQW])
        nc.gpsimd.dma_start(wvs[:], wv_ap[:])
        vna4 = vna[:].rearrange("p (t h v) -> p t h v", t=NTT, h=NHL)
        nc.gpsimd.dma_start(vna4[:, :, :, HD:HD + 8], ones_ap[:])
        nc.gpsimd.dma_start(xts[:, 3 * NCH * TQW:4 * NCH * TQW],
                            xt_ap[:, 3 * NCH * TQW:4 * NCH * TQW])
        nc.gpsimd.dma_start(wos[:], wo_ap[:])

        pt_tiles = {}

        # ---------- emission primitives ----------
        def warm():
            wtile = sb.tile([128, 640], BF16, tag="warm")
            wjunk = sb.tile([128, 8], F32, tag="wjunk")
            nc.vector.memset(wtile[:], 0.0)
            wp = ps.tile([128, TQW], F32, tag="proj", bufs=2, name="warm_ps")
            for i in range(14):
                nc.tensor.matmul(wp[:], wtile[:, 0:128], wtile[:, 128:640],
                                 start=True, stop=True)
            nc.vector.tensor_copy(wjunk[:], wp[:, 0:8])
            # load the exp table while DMAs stream
            wact = sb.tile([128, 8], BF16, tag="wact")
            nc.scalar.activation(wact[:], wp[:, 0:8],
                                 mybir.ActivationFunctionType.Exp, scale=0.125)

        def qk_window(p, ty, w):
            wsb, dst, bias = ((wqs, qts[p], bqs[p]), (wks, kts[p], bks[p]))[ty]
            acc = ps.tile([128, TQW], F32, tag="proj", bufs=2, name=f"qk{p}{ty}{w}")
            for c in range(NCH):
                nc.tensor.matmul(
                    acc[:], wsb[:, (p * NCH + c) * 128:(p * NCH + c + 1) * 128],
                    xts[:, (w * NCH + c) * TQW:(w * NCH + c + 1) * TQW],
                    start=(c == 0), stop=(c == NCH - 1))
            nc.vector.tensor_scalar_add(dst[:, w * TQW:(w + 1) * TQW],
                                        acc[:], bias[:])

        def v_tile(tt):
            s, q = tt // 4, tt % 4
            acc = ps.tile([128, CLOC], F32, tag="proj", bufs=2, name=f"v{tt}")
            for c in range(NCH):
                base = (s * NCH + c) * TQW + q * 128
                nc.tensor.matmul(acc[:], xts[:, base:base + 128],
                                 wvs[:, c * CLOC:(c + 1) * CLOC],
                                 start=(c == 0), stop=(c == NCH - 1))
            base = tt * NHL * VSTR
            dst = vna[:, base:base + NHL * VSTR].rearrange("p (h d) -> p h d", h=NHL)
            nc.vector.tensor_copy(dst[:, :, 0:HD],
                                  acc[:].rearrange("p (h d) -> p h d", h=NHL))

        def st_slot(p, w, g, h):
            qt, kt = qts[p], kts[p]
            nchunks = 4 * (w + 1)
            c0 = 2 * g
            st = ps.tile([128, 1024], F32, tag="st", bufs=2,
                         name=f"st{p}{w}{g}{h}")
            for j in range(2):
                c = c0 + j
                nc.tensor.matmul(
                    st[:, j * TQW:(j + 1) * TQW],
                    kt[h * 64:(h + 1) * 64, c * 128:(c + 1) * 128],
                    qt[h * 64:(h + 1) * 64, w * TQW:(w + 1) * TQW],
                    start=True, stop=True)
            pt = pt_pool.tile([128, 1024], BF16, tag="pt", name=f"pt{p}{w}{g}{h}")
            nc.scalar.activation(pt[:], st[:], mybir.ActivationFunctionType.Exp,
                                 scale=0.125)
            if c0 + 1 >= nchunks - 4:
                # causal mask: diag 128-block at cols [128*jp, 128*jp+128),
                # left of it = 0
                for j in range(2):
                    jp = (c0 + j) - 4 * w
                    if jp > 0:
                        nc.gpsimd.memset(pt[:, j * TQW: j * TQW + 128 * jp], 0.0)
                    dslc = pt[:, j * TQW + 128 * jp: j * TQW + 128 * jp + 128]
                    nc.gpsimd.tensor_mul(dslc, dslc, tri[:])
            pt_tiles[(p, w, g, h)] = pt

        def pv_group(p, w, g, h, accs):
            nchunks = 4 * (w + 1)
            c0 = 2 * g
            pt = pt_tiles.pop((p, w, g, h))
            for j in range(2):
                c = c0 + j
                vbase = c * NHL * VSTR + (2 * p + h) * VSTR
                nc.tensor.matmul(
                    accs[h][0:VSTR, :],
                    vna[:, vbase:vbase + VSTR],
                    pt[:, j * TQW:(j + 1) * TQW],
                    start=(c0 == 0 and j == 0),
                    stop=(c0 == nchunks - 2 and j == 1))

        def pv_tail(p, w, h, accs):
            # y (unnormalized) to SBUF; denominator row to lane 2p+h of dsb
            nc.vector.tensor_copy(yts[p][h * 64:(h + 1) * 64, w * TQW:(w + 1) * TQW],
                                  accs[h][0:HD, :])
            lane = 32 * (2 * p + h)
            nc.vector.tensor_copy(dsb[lane:lane + 1, :], accs[h][HD:HD + 1, :])

        def recip_window(w, half=None):
            # 8 cyc/elem iterative divide on DVE; cost scales with free dim
            sl = slice(0, TQW) if half is None else \
                slice(half * 256, half * 256 + 256)
            nc.vector.reciprocal(rcoll[:, sl], dsb[:, sl])

        def norm_rest(w, half=None):
            sl = slice(0, TQW) if half is None else \
                slice(half * 256, half * 256 + 256)
            n = TQW if half is None else 256
            for p in range(2):
                R = ps.tile([128, TQW], F32, tag="proj", bufs=2, name=f"R{p}{w}{half}")
                nc.tensor.matmul(R[:, 0:n], selw[:, p * 128:(p + 1) * 128],
                                 rcoll[:, sl], start=True, stop=True)
                for h in range(2):
                    yslc = yts[p][h * 64:(h + 1) * 64,
                                  w * TQW + sl.start:w * TQW + sl.stop]
                    nc.vector.tensor_mul(yslc, yslc, R[h * 64:(h + 1) * 64, 0:n])

        def out_tt(tt, dq=None):
            po = ps.tile([128, 1024], F32, tag="st", bufs=2, name=f"po{tt}")
            for nh in range(2):
                for cc in range(2):
                    nc.tensor.matmul(po[:, nh * TQW:(nh + 1) * TQW],
                                     yts[cc][:, tt * 128:(tt + 1) * 128],
                                     wos[:, cc * C + nh * TQW: cc * C + nh * TQW + TQW],
                                     start=(cc == 0), stop=(cc == 1))
            ost = ost_pool.tile([128, 1024], BF16, tag="ost", name=f"o{tt}")
            nc.vector.tensor_copy(ost[:], po[:])
            deng = (nc.sync, nc.gpsimd)[tt % 2] if dq is None else dq
            deng.dma_start(out_ap[tt * 128:(tt + 1) * 128, :], ost[:])

        # ---------- schedule ----------
        filler = []          # closures of PE work to sprinkle into attention

        def filler_step(n=1):
            for _ in range(n):
                if filler:
                    filler.pop(0)()

        def attn_pair(p, w, per_slot=1):
            ngroups = 2 * (w + 1)
            accs = [ps.tile([128, TQW], F32, tag=f"acc{h}", bufs=1,
                            name=f"acc{p}{w}{h}") for h in range(2)]
            for g in range(ngroups + LAG):
                if g < ngroups:
                    st_slot(p, w, g, 0)
                    st_slot(p, w, g, 1)
                if g >= LAG:
                    gg = g - LAG
                    pv_group(p, w, gg, 0, accs)
                    pv_group(p, w, gg, 1, accs)
                filler_step(per_slot)
            for h in range(2):
                pv_tail(p, w, h, accs)
            if p == 1:
                # start the window's reciprocal as soon as dsb is complete;
                # norm_rest pops as filler a few us later in attn(0,w+1)
                if w < NW - 1:
                    recip_window(w)

        warm()
        # iteration 0: explicit ordering so nothing waits on late DMA
        qk_window(0, 0, 0)
        qk_window(0, 1, 0)
        st_slot(0, 0, 0, 0)
        st_slot(0, 0, 0, 1)
        qk_window(1, 0, 0)
        qk_window(1, 1, 0)
        for tt in range(4):
            v_tile(tt)
        st_slot(0, 0, 1, 0)
        st_slot(0, 0, 1, 1)
        accs0 = [ps.tile([128, TQW], F32, tag=f"acc{h}", bufs=1,
                         name=f"acc00{h}") for h in range(2)]
        for g in range(2):
            pv_group(0, 0, g, 0, accs0)
            pv_group(0, 0, g, 1, accs0)
        for h in range(2):
            pv_tail(0, 0, h, accs0)
        # projections for window 1 run inside pair-1's window-0 attention
        for p in range(2):
            for ty in range(2):
                filler.append(lambda p=p, ty=ty: qk_window(p, ty, 1))
        for tt in range(4, 8):
            filler.append(lambda tt=tt: v_tile(tt))
        attn_pair(1, 0, per_slot=2)
        filler_step(len(filler))
        recip_window(0)

        for w in range(1, NW):
            # filler for this window: projections for slice w+1 (if any),
            # norm of window w-1 (after its recip is done), out of w-1
            items = []
            if w + 1 < NW:
                for p in range(2):
                    for ty in range(2):
                        items.append(lambda p=p, ty=ty, w=w: qk_window(p, ty, w + 1))
                for tt in range(4 * (w + 1), 4 * (w + 1) + 4):
                    items.append(lambda tt=tt: v_tile(tt))
            # norm after 2 proj slots (~2.5us after recip started)
            items.insert(min(2, len(items)), lambda w=w: norm_rest(w - 1))
            pos = 4 if len(items) > 4 else len(items)
            for i, tt in enumerate(range(4 * (w - 1), 4 * (w - 1) + 4)):
                items.insert(pos + 2 * i, lambda tt=tt: out_tt(tt))
            filler.extend(items)
            attn_pair(0, w, per_slot=1)
            attn_pair(1, w, per_slot=2)
            filler_step(len(filler))

        # tail: window 3 normalized + projected in tq halves to pipeline
        # recip / norm / out / DMA; scalar queue is free for DMA here
        recip_window(3, half=0)
        recip_window(3, half=1)
        norm_rest(3, half=0)
        out_tt(12, dq=nc.sync)
        out_tt(13, dq=nc.gpsimd)
        norm_rest(3, half=1)
        out_tt(14, dq=nc.scalar)
        out_tt(15, dq=nc.sync)

    nc.compile()
    return nc


def _selw():
    s = np.zeros((128, 2 * 128), np.float32)
    for p in range(2):
        for h in range(2):
            lane = 32 * (2 * p + h)
            s[lane, p * 128 + h * 64: p * 128 + h * 64 + 64] = 1.0
    return s


def _to_sbuf_chunks(a, nch):
    """[nch*128, F] row-major -> [128, nch*F] SBUF-native layout."""
    n, fdim = a.shape
    assert n == nch * 128
    return np.ascontiguousarray(
        a.reshape(nch, 128, fdim).transpose(1, 0, 2).reshape(128, nch * fdim))


def _prep_core_inputs(b, g, x, Wq, bq, Wk, bk, Wv, bv, Wo, bo):
    bf = ml_dtypes.bfloat16
    f = np.float32
    # x[b].T -> [C,T]; slice-major: [128, (s*NCH+c)*TQW + t']
    xtc = np.ascontiguousarray(x[b].T, dtype=f)          # [C, T]
    xt = (xtc.reshape(NCH, 128, NW, TQW).transpose(1, 2, 0, 3)
          .reshape(128, NW * NCH * TQW)).astype(bf)
    def pack(W, bvec):
        cols = []
        bp = np.empty((2, 128, 1), f)
        for p in range(2):
            h0, h1 = 4 * g + 2 * p, 4 * g + 2 * p + 1
            Wp = np.concatenate([W[:, h0 * HD:(h0 + 1) * HD],
                                 W[:, h1 * HD:(h1 + 1) * HD]], axis=1)
            cols.append(_to_sbuf_chunks(np.ascontiguousarray(Wp, f), NCH))
            bp[p, 0:64, 0] = bvec[h0 * HD:(h0 + 1) * HD]
            bp[p, 64:128, 0] = bvec[h1 * HD:(h1 + 1) * HD]
        return np.concatenate(cols, axis=1).astype(bf), bp
    wq, bqp = pack(Wq, bq)
    wk, bkp = pack(Wk, bk)
    wv = _to_sbuf_chunks(
        np.ascontiguousarray(Wv[:, g * CLOC:(g + 1) * CLOC], f), NCH).astype(bf)
    wo = _to_sbuf_chunks(
        np.ascontiguousarray(Wo[g * CLOC:(g + 1) * CLOC, :], f), 2).astype(bf)
    return {"xt": xt, "wq": wq, "wk": wk, "wv": wv, "wo": wo,
            "bq": bqp, "bk": bkp,
            "ones": np.ones((128, NTT, NHL, 8), bf),
            "selw": _selw(),
            "tri": np.triu(np.ones((128, 128))).astype(bf)}


def _run(inputs, trace=False, tmpdir=None):
    if "nc" not in _cache:
        _cache["nc"] = _build()
    nc = _cache["nc"]
    args = [np.asarray(inputs[k], np.float32) for k in
            ("x", "Wq", "bq", "Wk", "bk", "Wv", "bv", "Wo", "bo")]
    x, Wq, bq, Wk, bk, Wv, bv, Wo, bo = args
    in_maps = [_prep_core_inputs(c // 4, c % 4, x, Wq, bq, Wk, bk, Wv, bv, Wo, bo)
               for c in range(8)]
    res = bass_utils.run_bass_kernel_spmd(nc, in_maps, core_ids=list(range(8)),
                                          trace=trace, tmpdir=tmpdir)
    corr = (bv.astype(np.float64) @ Wo.astype(np.float64) + bo).astype(np.float64)
    out = np.empty((B, T, C), np.float32)
    for b in range(B):
        acc = np.zeros((T, C), np.float64)
        for g in range(4):
            acc += res.results[b * 4 + g]["out"].astype(np.float64)
        out[b] = (acc + corr).astype(np.float32)
    return out, res


def kernel(x, Wq, bq, Wk, bk, Wv, bv, Wo, bo):
    out, _ = _run(dict(x=x, Wq=Wq, bq=bq, Wk=Wk, bk=bk, Wv=Wv, bv=bv,
                       Wo=Wo, bo=bo))
    return out


def run_profiled(x, Wq, bq, Wk, bk, Wv, bv, Wo, bo, tmpdir=None):
    out, res = _run(dict(x=x, Wq=Wq, bq=bq, Wk=Wk, bk=bk, Wv=Wv, bv=bv,
                         Wo=Wo, bo=bo), trace=True, tmpdir=tmpdir)
    return out, res.exec_time_ns, res


# revision 11
# speedup vs baseline: 1.1596x; 1.1596x over previous
"""Causal self-attention kernel for Trainium2, 8-way sharded.

Problem: B=2, T=2048, C=1024, NH=16, hd=64. fp32 in/out.

Sharding: core = (batch b, head-group g of 4 heads). Each core computes its
4 heads' attention for its batch plus the partial output projection
y_local @ Wo[g*256:(g+1)*256, :]; the host sums the 4 partials per batch
(biases bv/bo are folded in exactly via a host-side correction row).

v2 schedule (vs baseline):
  - All matmul tensors bf16 (x, weights, q/k, P, V, y): halves DMA both
    ways, enables FWL weight loads. Scores/projections accumulate fp32 in
    PSUM so only operand quantization is lost (~0.5%, budget is 2e-2).
  - x is DMAed T-slice-major (slice = 512 tq x all 8 C-chunks = 0.5MB)
    so projections stream as slices land; PE is busy from ~4us and HAM
    stays warm instead of idling 24us for the full-x DMA.
  - Window-major over head-pairs: for each tq window w, both pairs' score/
    exp/PV run back-to-back, so the ScalarE exp stream never starves
    (baseline had a 42us ScalarE hole between pairs).
  - Per-window normalization: denominators for the 4 (pair,head) lanes of
    window w are batched into one [4,512] DVE reciprocal; the output
    projection for window w then runs as PE filler inside window w+1's
    (ACT-bound) attention instead of a 64us serialized tail.
  - Output DMAed as bf16 partials; host sums in float64.
"""
import contextlib

import ml_dtypes
import numpy as np

import concourse.bass as bass
import concourse.tile as tile
from concourse import bacc, mybir
from concourse import bass_utils

bass_utils.upload_artifacts = lambda tmpdir: "local://skipped"

B, T, C = 2, 2048, 1024
NH, HD = 16, 64
NHL = 4            # heads per core
CLOC = NHL * HD    # 256 local channels
NCH = C // 128     # 8 contraction chunks
TQW = 512          # tq window / T-slice width
NW = T // TQW      # 4 windows
NTT = T // 128     # 16 t-tiles / tk-chunks
VSTR = HD + 8      # 72: v cols per head + 8 ones cols (denom at row 64)
LAG = 2            # PV trails S^T by this many chunk-groups
F32R = mybir.dt.float32r
F32 = mybir.dt.float32
BF16 = mybir.dt.bfloat16

_cache = {}


def _build():
    nc = bacc.Bacc("TRN2", target_bir_lowering=False, debug=False, num_devices=8)

    # x slice-major: slice s holds chunks c at offset (s*NCH+c)*TQW
    xt_ap = nc.dram_tensor("xt", [128, NW * NCH * TQW], BF16, kind="ExternalInput").ap()
    wq_ap = nc.dram_tensor("wq", [128, 2 * NCH * 128], BF16, kind="ExternalInput").ap()
    wk_ap = nc.dram_tensor("wk", [128, 2 * NCH * 128], BF16, kind="ExternalInput").ap()
    wv_ap = nc.dram_tensor("wv", [128, NCH * CLOC], BF16, kind="ExternalInput").ap()
    wo_ap = nc.dram_tensor("wo", [128, 2 * C], BF16, kind="ExternalInput").ap()
    bq_ap = nc.dram_tensor("bq", [2, 128, 1], F32, kind="ExternalInput").ap()
    bk_ap = nc.dram_tensor("bk", [2, 128, 1], F32, kind="ExternalInput").ap()
    ones_ap = nc.dram_tensor("ones", [128, NTT, NHL, 8], BF16, kind="ExternalInput").ap()
    selw_ap = nc.dram_tensor("selw", [128, 2 * 128], F32, kind="ExternalInput").ap()
    tri_ap = nc.dram_tensor("tri", [128, 128], BF16, kind="ExternalInput").ap()
    out_ap = nc.dram_tensor("out", [T, C], BF16, kind="ExternalOutput").ap()

    with tile.TileContext(nc) as tc, contextlib.ExitStack() as ctx:
        sb = ctx.enter_context(tc.tile_pool(name="sb", bufs=1))
        ost_pool = ctx.enter_context(tc.tile_pool(name="ost", bufs=3))
        pt_pool = ctx.enter_context(tc.tile_pool(name="ptp", bufs=10))
        ps = ctx.enter_context(tc.tile_pool(name="ps", bufs=1, space="PSUM"))

        # ---- persistent SBUF tensors ----
        wqs = sb.tile([128, 2 * NCH * 128], BF16, tag="wqs")
        wks = sb.tile([128, 2 * NCH * 128], BF16, tag="wks")
        wvs = sb.tile([128, NCH * CLOC], BF16, tag="wvs")
        wos = sb.tile([128, 2 * C], BF16, tag="wos")
        xts = sb.tile([128, NW * NCH * TQW], BF16, tag="xts")
        qts = [sb.tile([128, T], BF16, tag=f"qt{p}", name=f"qt{p}") for p in range(2)]
        kts = [sb.tile([128, T], BF16, tag=f"kt{p}", name=f"kt{p}") for p in range(2)]
        vna = sb.tile([128, NTT * NHL * VSTR], BF16, tag="vna")
        yts = [sb.tile([128, T], BF16, tag=f"yt{p}", name=f"yt{p}") for p in range(2)]
        bqs = [sb.tile([128, 1], F32, tag=f"bq{p}", name=f"bqs{p}") for p in range(2)]
        bks = [sb.tile([128, 1], F32, tag=f"bk{p}", name=f"bks{p}") for p in range(2)]
        selw = sb.tile([128, 2 * 128], F32, tag="selw")
        dsb = sb.tile([128, TQW], F32, tag="dsb")
        nc.vector.memset(dsb[:], 1.0)
        tri = sb.tile([128, 128], BF16, tag="tri")
        rcoll = sb.tile([128, TQW], F32, tag="rcoll")

        # ---- input DMAs ----
        # scalar queue: tiny constants + w weights (all land < ~5us, before
        # the exp stream owns ScalarE). sync: x slices 0,1. gpsimd: wv/ones
        # then x slices 2,3 and wo.
        for p in range(2):
            nc.scalar.dma_start(bqs[p][:], bq_ap[p])
            nc.scalar.dma_start(bks[p][:], bk_ap[p])
        HW = NCH * 128  # 1024 cols = one pair's weight block
        nc.scalar.dma_start(wqs[:, 0:HW], wq_ap[:, 0:HW])
        nc.scalar.dma_start(wks[:, 0:HW], wk_ap[:, 0:HW])
        nc.scalar.dma_start(wqs[:, HW:2 * HW], wq_ap[:, HW:2 * HW])
        nc.scalar.dma_start(wks[:, HW:2 * HW], wk_ap[:, HW:2 * HW])
        nc.scalar.dma_start(selw[:], selw_ap[:])
        nc.scalar.dma_start(tri[:], tri_ap[:])
        nc.sync.dma_start(xts[:, 0:NCH * TQW], xt_ap[:, 0:NCH * TQW])
        nc.sync.dma_start(xts[:, NCH * TQW:2 * NCH * TQW],
                          xt_ap[:, NCH * TQW:2 * NCH * TQW])
        nc.sync.dma_start(xts[:, 2 * NCH * TQW:3 * NCH * TQW],
                          xt_ap[:, 2 * NCH * TQW:3 * NCH * TQW])
        nc.gpsimd.dma_start(wvs[:], wv_ap[:])
        vna4 = vna[:].rearrange("p (t h v) -> p t h v", t=NTT, h=NHL)
        nc.gpsimd.dma_start(vna4[:, :, :, HD:HD + 8], ones_ap[:])
        nc.gpsimd.dma_start(xts[:, 3 * NCH * TQW:4 * NCH * TQW],
                            xt_ap[:, 3 * NCH * TQW:4 * NCH * TQW])
        nc.gpsimd.dma_start(wos[:], wo_ap[:])

        pt_tiles = {}

        # ---------- emission primitives ----------
        def warm():
            wtile = sb.tile([128, 640], BF16, tag="warm")
            wjunk = sb.tile([128, 8], F32, tag="wjunk")
            nc.vector.memset(wtile[:], 0.0)
            wp = ps.tile([128, TQW], F32, tag="proj", bufs=2, name="warm_ps")
            for i in range(14):
                nc.tensor.matmul(wp[:], wtile[:, 0:128], wtile[:, 128:640],
                                 start=True, stop=True)
            nc.vector.tensor_copy(wjunk[:], wp[:, 0:8])
            # load the exp table while DMAs stream
            wact = sb.tile([128, 8], BF16, tag="wact")
            nc.scalar.activation(wact[:], wp[:, 0:8],
                                 mybir.ActivationFunctionType.Exp, scale=0.125)

        def qk_window(p, ty, w):
            wsb, dst, bias = ((wqs, qts[p], bqs[p]), (wks, kts[p], bks[p]))[ty]
            acc = ps.tile([128, TQW], F32, tag="proj", bufs=2, name=f"qk{p}{ty}{w}")
            for c in range(NCH):
                nc.tensor.matmul(
                    acc[:], wsb[:, (p * NCH + c) * 128:(p * NCH + c + 1) * 128],
                    xts[:, (w * NCH + c) * TQW:(w * NCH + c + 1) * TQW],
                    start=(c == 0), stop=(c == NCH - 1))
            nc.vector.tensor_scalar_add(dst[:, w * TQW:(w + 1) * TQW],
                                        acc[:], bias[:])

        def v_tile(tt):
            s, q = tt // 4, tt % 4
            acc = ps.tile([128, CLOC], F32, tag="proj", bufs=2, name=f"v{tt}")
            for c in range(NCH):
                base = (s * NCH + c) * TQW + q * 128
                nc.tensor.matmul(acc[:], xts[:, base:base + 128],
                                 wvs[:, c * CLOC:(c + 1) * CLOC],
                                 start=(c == 0), stop=(c == NCH - 1))
            base = tt * NHL * VSTR
            dst = vna[:, base:base + NHL * VSTR].rearrange("p (h d) -> p h d", h=NHL)
            nc.vector.tensor_copy(dst[:, :, 0:HD],
                                  acc[:].rearrange("p (h d) -> p h d", h=NHL))

        def st_slot(p, w, g, h):
            qt, kt = qts[p], kts[p]
            nchunks = 4 * (w + 1)
            c0 = 2 * g
            st = ps.tile([128, 1024], F32, tag="st", bufs=2,
                         name=f"st{p}{w}{g}{h}")
            for j in range(2):
                c = c0 + j
                nc.tensor.matmul(
                    st[:, j * TQW:(j + 1) * TQW],
                    kt[h * 64:(h + 1) * 64, c * 128:(c + 1) * 128],
                    qt[h * 64:(h + 1) * 64, w * TQW:(w + 1) * TQW],
                    start=True, stop=True)
            pt = pt_pool.tile([128, 1024], BF16, tag="pt", name=f"pt{p}{w}{g}{h}")
            nc.scalar.activation(pt[:], st[:], mybir.ActivationFunctionType.Exp,
                                 scale=0.125)
            if c0 + 1 >= nchunks - 4:
                # causal mask: diag 128-block at cols [128*jp, 128*jp+128),
                # left of it = 0
                for j in range(2):
                    jp = (c0 + j) - 4 * w
                    if jp > 0:
                        nc.gpsimd.memset(pt[:, j * TQW: j * TQW + 128 * jp], 0.0)
                    dslc = pt[:, j * TQW + 128 * jp: j * TQW + 128 * jp + 128]
                    nc.vector.tensor_mul(dslc, dslc, tri[:])
            pt_tiles[(p, w, g, h)] = pt

        def pv_group(p, w, g, h, accs):
            nchunks = 4 * (w + 1)
            c0 = 2 * g
            pt = pt_tiles.pop((p, w, g, h))
            for j in range(2):
                c = c0 + j
                vbase = c * NHL * VSTR + (2 * p + h) * VSTR
                nc.tensor.matmul(
                    accs[h][0:VSTR, :],
                    vna[:, vbase:vbase + VSTR],
                    pt[:, j * TQW:(j + 1) * TQW],
                    start=(c0 == 0 and j == 0),
                    stop=(c0 == nchunks - 2 and j == 1))

        def pv_tail(p, w, h, accs):
            # y (unnormalized) to SBUF; denominator row to lane 2p+h of dsb
            nc.vector.tensor_copy(yts[p][h * 64:(h + 1) * 64, w * TQW:(w + 1) * TQW],
                                  accs[h][0:HD, :])
            lane = 32 * (2 * p + h)
            nc.vector.tensor_copy(dsb[lane:lane + 1, :], accs[h][HD:HD + 1, :])

        def recip_window(w, half=None):
            # 8 cyc/elem iterative divide on DVE; cost scales with free dim
            sl = slice(0, TQW) if half is None else \
                slice(half * 256, half * 256 + 256)
            nc.vector.reciprocal(rcoll[:, sl], dsb[:, sl])

        def norm_rest(w, half=None):
            sl = slice(0, TQW) if half is None else \
                slice(half * 256, half * 256 + 256)
            n = TQW if half is None else 256
            for p in range(2):
                R = ps.tile([128, TQW], F32, tag="proj", bufs=2, name=f"R{p}{w}{half}")
                nc.tensor.matmul(R[:, 0:n], selw[:, p * 128:(p + 1) * 128],
                                 rcoll[:, sl], start=True, stop=True)
                for h in range(2):
                    yslc = yts[p][h * 64:(h + 1) * 64,
                                  w * TQW + sl.start:w * TQW + sl.stop]
                    nc.vector.tensor_mul(yslc, yslc, R[h * 64:(h + 1) * 64, 0:n])

        def out_tt(tt, dq=None):
            po = ps.tile([128, 1024], F32, tag="st", bufs=2, name=f"po{tt}")
            for nh in range(2):
                for cc in range(2):
                    nc.tensor.matmul(po[:, nh * TQW:(nh + 1) * TQW],
                                     yts[cc][:, tt * 128:(tt + 1) * 128],
                                     wos[:, cc * C + nh * TQW: cc * C + nh * TQW + TQW],
                                     start=(cc == 0), stop=(cc == 1))
            ost = ost_pool.tile([128, 1024], BF16, tag="ost", name=f"o{tt}")
            nc.vector.tensor_copy(ost[:], po[:])
            deng = nc.sync if dq is None else dq
            deng.dma_start(out_ap[tt * 128:(tt + 1) * 128, :], ost[:])

        # ---------- schedule ----------
        filler = []          # closures of PE work to sprinkle into attention

        def filler_step(n=1):
            for _ in range(n):
                if filler:
                    filler.pop(0)()

        def attn_pair(p, w, per_slot=1):
            ngroups = 2 * (w + 1)
            accs = [ps.tile([128, TQW], F32, tag=f"acc{h}", bufs=1,
                            name=f"acc{p}{w}{h}") for h in range(2)]
            for g in range(ngroups + LAG):
                if g < ngroups:
                    st_slot(p, w, g, 0)
                    st_slot(p, w, g, 1)
                if g >= LAG:
                    gg = g - LAG
                    pv_group(p, w, gg, 0, accs)
                    pv_group(p, w, gg, 1, accs)
                filler_step(per_slot)
            for h in range(2):
                pv_tail(p, w, h, accs)
            if p == 1:
                # start the window's reciprocal as soon as dsb is complete;
                # norm_rest pops as filler a few us later in attn(0,w+1)
                if w < NW - 1:
                    recip_window(w)

        warm()
        # iteration 0: explicit ordering so nothing waits on late DMA
        qk_window(0, 0, 0)
        qk_window(0, 1, 0)
        st_slot(0, 0, 0, 0)
        st_slot(0, 0, 0, 1)
        qk_window(1, 0, 0)
        qk_window(1, 1, 0)
        for tt in range(4):
            v_tile(tt)
        st_slot(0, 0, 1, 0)
        st_slot(0, 0, 1, 1)
        accs0 = [ps.tile([128, TQW], F32, tag=f"acc{h}", bufs=1,
                         name=f"acc00{h}") for h in range(2)]
        for g in range(2):
            pv_group(0, 0, g, 0, accs0)
            pv_group(0, 0, g, 1, accs0)
        for h in range(2):
            pv_tail(0, 0, h, accs0)
        # projections for window 1 run inside pair-1's window-0 attention
        for p in range(2):
            for ty in range(2):
                filler.append(lambda p=p, ty=ty: qk_window(p, ty, 1))
        for tt in range(4, 8):
            filler.append(lambda tt=tt: v_tile(tt))
        attn_pair(1, 0, per_slot=2)  # proj(1) must finish before attn(0,1)
        filler_step(len(filler))
        recip_window(0)

        for w in range(1, NW):
            # filler for this window: projections for slice w+1 (if any),
            # norm of window w-1 (after its recip is done), out of w-1
            items = []
            if w + 1 < NW:
                for p in range(2):
                    for ty in range(2):
                        items.append(lambda p=p, ty=ty, w=w: qk_window(p, ty, w + 1))
                for tt in range(4 * (w + 1), 4 * (w + 1) + 4):
                    items.append(lambda tt=tt: v_tile(tt))
            # norm after 2 proj slots (~2.5us after recip started)
            items.insert(min(2, len(items)), lambda w=w: norm_rest(w - 1))
            pos = 4 if len(items) > 4 else len(items)
            for i, tt in enumerate(range(4 * (w - 1), 4 * (w - 1) + 4)):
                items.insert(pos + 2 * i, lambda tt=tt: out_tt(tt))
            filler.extend(items)
            attn_pair(0, w, per_slot=1)
            attn_pair(1, w, per_slot=1)
            filler_step(len(filler))

        # tail: window 3 normalized + projected in tq halves to pipeline
        # recip / norm / out / DMA; scalar queue is free for DMA here
        recip_window(3, half=0)
        recip_window(3, half=1)
        norm_rest(3, half=0)
        out_tt(12, dq=nc.sync)
        out_tt(13, dq=nc.gpsimd)
        norm_rest(3, half=1)
        out_tt(14, dq=nc.scalar)
        out_tt(15, dq=nc.sync)

    nc.compile()
    return nc


def _selw():
    s = np.zeros((128, 2 * 128), np.float32)
    for p in range(2):
        for h in range(2):
            lane = 32 * (2 * p + h)
            s[lane, p * 128 + h * 64: p * 128 + h * 64 + 64] = 1.0
    return s


def _to_sbuf_chunks(a, nch):
    """[nch*128, F] row-major -> [128, nch*F] SBUF-native layout."""
    n, fdim = a.shape
    assert n == nch * 128
    return np.ascontiguousarray(
        a.reshape(nch, 128, fdim).transpose(1, 0, 2).reshape(128, nch * fdim))


def _prep_core_inputs(b, g, x, Wq, bq, Wk, bk, Wv, bv, Wo, bo):
    bf = ml_dtypes.bfloat16
    f = np.float32
    # x[b].T -> [C,T]; slice-major: [128, (s*NCH+c)*TQW + t']
    xtc = np.ascontiguousarray(x[b].T, dtype=f)          # [C, T]
    xt = (xtc.reshape(NCH, 128, NW, TQW).transpose(1, 2, 0, 3)
          .reshape(128, NW * NCH * TQW)).astype(bf)
    def pack(W, bvec):
        cols = []
        bp = np.empty((2, 128, 1), f)
        for p in range(2):
            h0, h1 = 4 * g + 2 * p, 4 * g + 2 * p + 1
            Wp = np.concatenate([W[:, h0 * HD:(h0 + 1) * HD],
                                 W[:, h1 * HD:(h1 + 1) * HD]], axis=1)
            cols.append(_to_sbuf_chunks(np.ascontiguousarray(Wp, f), NCH))
            bp[p, 0:64, 0] = bvec[h0 * HD:(h0 + 1) * HD]
            bp[p, 64:128, 0] = bvec[h1 * HD:(h1 + 1) * HD]
        return np.concatenate(cols, axis=1).astype(bf), bp
    wq, bqp = pack(Wq, bq)
    wk, bkp = pack(Wk, bk)
    wv = _to_sbuf_chunks(
        np.ascontiguousarray(Wv[:, g * CLOC:(g + 1) * CLOC], f), NCH).astype(bf)
    wo = _to_sbuf_chunks(
        np.ascontiguousarray(Wo[g * CLOC:(g + 1) * CLOC, :], f), 2).astype(bf)
    return {"xt": xt, "wq": wq, "wk": wk, "wv": wv, "wo": wo,
            "bq": bqp, "bk": bkp,
            "ones": np.ones((128, NTT, NHL, 8), bf),
            "selw": _selw(),
            "tri": np.triu(np.ones((128, 128))).astype(bf)}


def _run(inputs, trace=False, tmpdir=None):
    if "nc" not in _cache:
        _cache["nc"] = _build()
    nc = _cache["nc"]
    args = [np.asarray(inputs[k], np.float32) for k in
            ("x", "Wq", "bq", "Wk", "bk", "Wv", "bv", "Wo", "bo")]
    x, Wq, bq, Wk, bk, Wv, bv, Wo, bo = args
    in_maps = [_prep_core_inputs(c // 4, c % 4, x, Wq, bq, Wk, bk, Wv, bv, Wo, bo)
               for c in range(8)]
    res = bass_utils.run_bass_kernel_spmd(nc, in_maps, core_ids=list(range(8)),
                                          trace=trace, tmpdir=tmpdir)
    corr = (bv.astype(np.float64) @ Wo.astype(np.float64) + bo).astype(np.float64)
    out = np.empty((B, T, C), np.float32)
    for b in range(B):
        acc = np.zeros((T, C), np.float64)
        for g in range(4):
            acc += res.results[b * 4 + g]["out"].astype(np.float64)
        out[b] = (acc + corr).astype(np.float32)
    return out, res


def kernel(x, Wq, bq, Wk, bk, Wv, bv, Wo, bo):
    out, _ = _run(dict(x=x, Wq=Wq, bq=bq, Wk=Wk, bk=bk, Wv=Wv, bv=bv,
                       Wo=Wo, bo=bo))
    return out


def run_profiled(x, Wq, bq, Wk, bk, Wv, bv, Wo, bo, tmpdir=None):
    out, res = _run(dict(x=x, Wq=Wq, bq=bq, Wk=Wk, bk=bk, Wv=Wv, bv=bv,
                         Wo=Wo, bo=bo), trace=True, tmpdir=tmpdir)
    return out, res.exec_time_ns, res
